# revision 1
# baseline (speedup 1.0000x reference)
"""Fused EmbeddingBag(mean) + Linear kernel for Trainium2, 8-core data-parallel.

Strategy: batch is sharded 8 ways (2048 bags/core). The embedding table gets a
host-appended zero row; invalid (beyond-length) token slots are redirected to it
on-device, so the length-masked sum becomes a plain sum. Per 128-bag tile, one
indirect DMA gathers all 6400 token rows (each partition = one bag's 50
embeddings), a strided-AP vector reduce sums over the 50 slots, and a single
matmul against [W.T; b; null_emb] applies projection, bias, and the
empty-bag null-embedding select in one shot.
"""

import sys

sys.path.insert(0, "/opt/trn_rl_repo")

from contextlib import ExitStack

import numpy as np

import concourse.bass as bass
import concourse.bacc as bacc
import concourse.mybir as mybir
import concourse.tile as tile
from concourse.bass import IndirectOffsetOnAxis
from concourse.masks import make_identity

VOCAB, EMBED, COND = 100000, 64, 256
B, L = 16384, 50
NCORES = 8
BLOC = B // NCORES  # 2048 bags per core
P = 128
NT = BLOC // P  # 16 tiles per core

F32 = mybir.dt.float32
I32 = mybir.dt.int32


def build_nc(g_bufs: int = 3) -> bass.Bass:
    nc = bacc.Bacc("TRN2", target_bir_lowering=False)

    ids = nc.declare_dram_parameter("ids", [BLOC, L + 1], I32, isOutput=False)
    emb = nc.declare_dram_parameter("emb", [VOCAB + 1, EMBED], F32, isOutput=False)
    wext = nc.declare_dram_parameter("wext", [EMBED + 2, COND], F32, isOutput=False)
    out = nc.declare_dram_parameter("out", [BLOC, COND], F32, isOutput=True)

    op = mybir.AluOpType

    with tile.TileContext(nc) as tc, ExitStack() as ctx:
        const = ctx.enter_context(tc.tile_pool(name="const", bufs=1))
        sb = ctx.enter_context(tc.tile_pool(name="sb", bufs=6))
        gp = ctx.enter_context(tc.tile_pool(name="gp", bufs=g_bufs))
        ps = ctx.enter_context(tc.tile_pool(name="ps", bufs=2, space="PSUM"))

        # One-time constants
        idt = const.tile([P, P], F32, tag="idt")
        make_identity(nc, idt[:])
        iota_l = const.tile([P, L], I32, tag="iota")
        nc.gpsimd.iota(out=iota_l[:], pattern=[[1, L]], base=0, channel_multiplier=0)
        bigc = const.tile([P, L], I32, tag="bigc")
        nc.gpsimd.memset(bigc[:], VOCAB)  # index of the all-zero row
        wext_sb = const.tile([EMBED + 2, COND], F32, tag="wext")
        nc.gpsimd.dma_start(out=wext_sb[:], in_=wext[:])

        for t in range(NT):
            rows = slice(t * P, (t + 1) * P)

            ids_t = sb.tile([P, L + 1], I32, tag="ids")
            nc.gpsimd.dma_start(out=ids_t[:], in_=ids[rows, :])

            lenf = sb.tile([P, 1], F32, tag="lenf")
            nc.vector.tensor_copy(out=lenf[:], in_=ids_t[:, L : L + 1])

            # mask[p, l] = l < len[p]; invalid slots -> zero-row index
            mask_t = sb.tile([P, L], I32, tag="mask")
            nc.vector.tensor_scalar(
                out=mask_t[:], in0=iota_l[:], scalar1=lenf[:, :1], scalar2=None,
                op0=op.is_lt,
            )
            idx_t = sb.tile([P, L], I32, tag="idx")
            nc.vector.select(
                out=idx_t[:], mask=mask_t[:], on_true=ids_t[:, 0:L], on_false=bigc[:]
            )

            # Gather all 50 embeddings per bag: partition p gets bag t*128+p.
            g_t = gp.tile([P, L * EMBED], F32, tag="g")
            for l in range(L):
                nc.gpsimd.indirect_dma_start(
                    out=g_t[:, l * EMBED : (l + 1) * EMBED],
                    out_offset=None,
                    in_=emb[:],
                    in_offset=IndirectOffsetOnAxis(ap=idx_t[:, l : l + 1], axis=0),
                )

            # Sum over the 50 slots (strided view [P, e, l], reduce innermost l)
            s_t = sb.tile([P, EMBED], F32, tag="s")
            nc.vector.tensor_reduce(
                out=s_t[:],
                in_=g_t[:].rearrange("p (l e) -> p e l", l=L, e=EMBED),
                axis=mybir.AxisListType.X,
                op=op.add,
            )

            # mean = sum / max(len, 1); flags for bias-vs-null selection
            den = sb.tile([P, 1], F32, tag="den")
            nc.vector.tensor_scalar_max(out=den[:], in0=lenf[:], scalar1=1.0)
            rec = sb.tile([P, 1], F32, tag="rec")
            nc.vector.reciprocal(out=rec[:], in_=den[:])

            tr = sb.tile([P, EMBED + 2], F32, tag="tr")
            nc.vector.tensor_scalar_mul(
                out=tr[:, 0:EMBED], in0=s_t[:], scalar1=rec[:, :1]
            )
            nc.vector.tensor_scalar(
                out=tr[:, EMBED : EMBED + 1], in0=lenf[:], scalar1=0.0, scalar2=None,
                op0=op.is_gt,
            )
            nc.vector.tensor_scalar(
                out=tr[:, EMBED + 1 : EMBED + 2], in0=lenf[:], scalar1=0.0,
                scalar2=None, op0=op.is_le,
            )

            # [P, 66] -> [66, P] so the projection contracts over E on partitions
            pT = ps.tile([EMBED + 2, P], F32, tag="pT", space="PSUM")
            nc.tensor.transpose(out=pT[:], in_=tr[:], identity=idt[:])
            mT = sb.tile([EMBED + 2, P], F32, tag="mT")
            nc.scalar.copy(out=mT[:], in_=pT[:])

            # out[128, 256] = meanT.T @ [W.T; b; null]: proj + bias + null select
            po = ps.tile([P, COND], F32, tag="po", space="PSUM")
            nc.tensor.matmul(out=po[:], lhsT=mT[:], rhs=wext_sb[:], start=True, stop=True)
            ob = sb.tile([P, COND], F32, tag="ob")
            nc.scalar.copy(out=ob[:], in_=po[:])
            nc.gpsimd.dma_start(out=out[rows, :], in_=ob[:])

    nc.compile()
    return nc


_NC_CACHE: dict = {}


def _get_nc(g_bufs: int = 3) -> bass.Bass:
    if g_bufs not in _NC_CACHE:
        _NC_CACHE[g_bufs] = build_nc(g_bufs)
    return _NC_CACHE[g_bufs]


def make_in_maps(token_ids, lengths, emb_table, W, b, null_emb):
    lens32 = np.asarray(lengths).astype(np.int32, copy=False).reshape(B, 1)
    ids32 = np.ascontiguousarray(
        np.concatenate(
            [np.asarray(token_ids).astype(np.int32, copy=False), lens32], axis=1
        )
    )
    emb_ext = np.concatenate(
        [np.asarray(emb_table, dtype=np.float32), np.zeros((1, EMBED), np.float32)]
    )
    wext = np.concatenate(
        [
            np.asarray(W, dtype=np.float32).T,  # [64, 256]
            np.asarray(b, dtype=np.float32)[None, :],
            np.asarray(null_emb, dtype=np.float32)[None, :],
        ]
    )  # [66, 256]
    return [
        {
            "ids": ids32[c * BLOC : (c + 1) * BLOC],
            "emb": emb_ext,
            "wext": wext,
        }
        for c in range(NCORES)
    ]


def kernel(token_ids, lengths, emb_table, W, b, null_emb, **run_kwargs):
    from concourse.bass_utils import run_bass_kernel_spmd

    nc = _get_nc()
    in_maps = make_in_maps(token_ids, lengths, emb_table, W, b, null_emb)
    res = run_bass_kernel_spmd(nc, in_maps, core_ids=list(range(NCORES)), **run_kwargs)
    out = np.concatenate([res.results[c]["out"] for c in range(NCORES)], axis=0)
    return out

